# revision 11
# baseline (speedup 1.0000x reference)
"""Cross-attention kernel for Trainium2, distributed over 8 NeuronCores.

Problem: B=4, Sk=4096, Sq=2048, d_model=1024, dims=64 (fp32 reference).

Sharding (hardcoded): core c -> (batch b = c//2, decoder half h = c%2).
Each core computes out[b, h*1024:(h+1)*1024, :] from enc[b] and its decoder
slice. No collectives.

Per-core dataflow (all layouts chosen so no large on-chip transposes are
needed):
  - Host pre-transposes/casts activations to bf16: encT [1024,4096],
    decT [1024,1024] (d_model on partitions).
  - KV^T projection: lhsT = [Wv | Wk] [128d, 128], rhs = encT chunks
    -> psum [128, 512] where rows 0:64 = V^T, 64:128 = K^T. Full PE array.
  - V^T is evacuated into an 80-partition tile whose row 64 is constant 1.0;
    a DMA x-bar transpose then yields V-natural blocks [128k, 80] whose
    col 64 is the ones column -> AV lhsT [128, 65] directly, and the ones
    column accumulates the softmax denominator during the AV matmul.
  - Scores computed transposed: S^T[k,q] = (K Q^T); exp(S^T) on ACT
    (PSUM->SBUF bf16) feeds the AV matmul as the moving operand. No
    max-subtraction (|scores| ~ N(0,1), exp is safe in fp32).
  - out^T [65, q] accumulated in PSUM over k blocks; final transpose of the
    small output via PE + per-partition reciprocal-scale on DVE.
"""

import numpy as np
import ml_dtypes

import concourse.bass as bass
import concourse.bacc as bacc
import concourse.tile as tile
from concourse import mybir
from concourse._compat import with_exitstack
from concourse.bass_utils import run_bass_kernel_spmd
from concourse.masks import make_identity

BF16 = mybir.dt.bfloat16
F32 = mybir.dt.float32

B, SK, SQ_FULL, D, DIMS = 4, 4096, 2048, 1024, 64
N_CORES = 8
SQ = SQ_FULL * B // N_CORES  # 1024 decoder rows per core
DC = D // 128  # d_model chunks of 128
KPAIRS = SK // 1024  # 4 enc column pair-tiles
KBLKS = SK // 128  # 32 k blocks for attention
OBLKS = SQ // 128  # 8 output row blocks


@with_exitstack
def _body(ctx, tc, encT, decT, wkv, wq, bv, bk, bq, out):
    nc = tc.nc

    singles = ctx.enter_context(tc.tile_pool(name="singles", bufs=1))
    loads = ctx.enter_context(tc.tile_pool(name="loads", bufs=3))
    ps_pool = ctx.enter_context(tc.tile_pool(name="ps", bufs=2, space="PSUM"))
    po_pool = ctx.enter_context(tc.tile_pool(name="po", bufs=2, space="PSUM"))
    pt_pool = ctx.enter_context(tc.tile_pool(name="pt", bufs=2, space="PSUM"))
    at_pool = ctx.enter_context(tc.tile_pool(name="at", bufs=4))
    outs = ctx.enter_context(tc.tile_pool(name="outs", bufs=3))

    # --- constants (small loads on the SWDGE/gpsimd queue, so the big
    # activation streams own the HWDGE path) ---
    wkv_sb = singles.tile([128, DC, 128], BF16)
    nc.gpsimd.dma_start(out=wkv_sb, in_=wkv.rearrange("(c p) m -> p c m", p=128))
    wq_sb = singles.tile([128, DC, DIMS], BF16)
    nc.gpsimd.dma_start(out=wq_sb, in_=wq.rearrange("(c p) m -> p c m", p=128))
    bv_sb = singles.tile([DIMS, 1], F32)
    nc.gpsimd.dma_start(out=bv_sb, in_=bv)
    bk_sb = singles.tile([DIMS, 1], F32)
    nc.gpsimd.dma_start(out=bk_sb, in_=bk)
    bq_sb = singles.tile([DIMS, 1], F32)
    nc.gpsimd.dma_start(out=bq_sb, in_=bq)
    ident = singles.tile([128, 128], F32)
    make_identity(nc, ident)

    # --- persistent activations ---
    # K^T stored in quadrant-alternating layout: even k-blocks on partitions
    # 0:64, odd k-blocks on partitions 64:128 (same 128-col block). Matching
    # q operands are duplicated on both partition halves. Consecutive S
    # matmuls then target disjoint PE-array row halves, so each LDWEIGHTS
    # overlaps the in-flight matmul instead of stalling behind it.
    kTd = singles.tile([128, SK // 2], BF16)
    # V^T with a ones row baked in at row 64 (rows 65:80 are never consumed;
    # partition count must be 16-aligned for the x-bar transpose).
    vTx = singles.tile([80, SK], BF16)
    nc.gpsimd.memset(vTx[64:65, :], 1.0)
    # V natural blocks: vnat[p, c, 0:64] = V[c*128+p, :], col 64 = 1.0
    vnat = singles.tile([128, KBLKS, 80], BF16)
    qTd = singles.tile([128, SQ], BF16)
    oT = singles.tile([DIMS + 1, SQ], F32)

    # --- activation loads: 1 MB chunks on HWDGE, ordered so the first
    # KV pair and the decoder land first ---
    esbs = []
    for kp in range(KPAIRS):
        e0 = loads.tile([128, 4, 1024], BF16, tag="eload", name=f"esb{kp}a")
        e1 = loads.tile([128, 4, 1024], BF16, tag="eload", name=f"esb{kp}b")
        esbs.append((e0, e1))
    dsb = loads.tile([128, DC, SQ], BF16, tag="dload")

    def load_pair(kp):
        enc_r = encT.rearrange("(c p) n -> p c n", p=128)
        sl = slice(kp * 1024, (kp + 1) * 1024)
        nc.sync.dma_start(out=esbs[kp][0], in_=enc_r[:, 0:4, sl])
        nc.sync.dma_start(out=esbs[kp][1], in_=enc_r[:, 4:8, sl])

    load_pair(0)
    dec_r = decT.rearrange("(c p) n -> p c n", p=128)
    nc.sync.dma_start(out=dsb[:, 0:4, :], in_=dec_r[:, 0:4, :])
    nc.sync.dma_start(out=dsb[:, 4:8, :], in_=dec_r[:, 4:8, :])
    for kp in range(1, KPAIRS):
        load_pair(kp)

    def kv_pair(kp):
        pskv = ps_pool.tile([128, 2, 512], F32, tag="ps", name=f"pskv{kp}")
        for d in range(DC):
            esb = esbs[kp][d // 4]
            for j in range(2):
                nc.tensor.matmul(
                    pskv[:, j, :], lhsT=wkv_sb[:, d, :],
                    rhs=esb[:, d % 4, j * 512:(j + 1) * 512],
                    start=(d == 0), stop=(d == DC - 1),
                )
        for j in range(2):
            sl = slice(kp * 1024 + j * 512, kp * 1024 + (j + 1) * 512)
            nc.vector.tensor_scalar_add(vTx[0:DIMS, sl], pskv[0:DIMS, j, :], bv_sb)
            # scatter K^T into the quadrant layout: chunk j holds k blocks
            # kb0..kb0+3; evens -> partitions 0:64, odds -> 64:128
            kb0 = kp * 8 + j * 4
            ksrc = pskv[DIMS:128, j, :].rearrange("p (b t c) -> p b t c", b=2, c=128)
            kdst = slice((kb0 // 2) * 128, (kb0 // 2 + 2) * 128)
            nc.vector.tensor_scalar_add(
                kTd[0:DIMS, kdst].rearrange("p (b c) -> p b c", c=128),
                ksrc[:, :, 0, :], bk_sb,
            )
            nc.vector.tensor_scalar_add(
                kTd[DIMS:128, kdst].rearrange("p (b c) -> p b c", c=128),
                ksrc[:, :, 1, :], bk_sb,
            )
        # V natural layout via one x-bar transpose per 1024 columns
        nc.sync.dma_start_transpose(
            out=vnat[:, kp * 8:(kp + 1) * 8, :],
            in_=vTx[:, kp * 1024:(kp + 1) * 1024],
        )

    # --- K^T / V^T projection (packed): rows 0:64 = V^T, 64:128 = K^T ---
    kv_pair(0)

    # --- Q^T projection: qT[64, SQ] = Wq^T @ decT ---
    psq = ps_pool.tile([DIMS, 2, 512], F32, tag="ps")
    for d in range(DC):
        for j in range(2):
            nc.tensor.matmul(
                psq[:, j, :], lhsT=wq_sb[:, d, :], rhs=dsb[:, d, j * 512:(j + 1) * 512],
                start=(d == 0), stop=(d == DC - 1),
            )
    for j in range(2):
        nc.vector.tensor_scalar_add(
            qTd[0:DIMS, j * 512:(j + 1) * 512], psq[:, j, :], bq_sb
        )
        nc.vector.tensor_scalar_add(
            qTd[DIMS:128, j * 512:(j + 1) * 512], psq[:, j, :], bq_sb
        )

    for kp in range(1, KPAIRS):
        kv_pair(kp)

    # --- attention: S^T = K Q^T per k block (both q tiles share the
    #     stationary operand); exp; accumulate [V|1]^T @ exp(S^T) ---
    po0 = po_pool.tile([DIMS + 1, 512], F32, tag="po")
    po1 = po_pool.tile([DIMS + 1, 512], F32, tag="po")
    pos = [po0, po1]
    for kb in range(KBLKS):
        hp = DIMS * (kb % 2)
        cb = kb // 2
        pss = ps_pool.tile([128, 2, 512], F32, tag="ps")
        for j in range(2):
            nc.tensor.matmul(
                pss[:, j, :], lhsT=kTd[hp:hp + DIMS, cb * 128:(cb + 1) * 128],
                rhs=qTd[hp:hp + DIMS, j * 512:(j + 1) * 512], start=True, stop=True,
            )
        at = at_pool.tile([128, 2, 512], BF16, tag="at")
        nc.scalar.activation(at, pss, mybir.ActivationFunctionType.Exp)
        for j in range(2):
            nc.tensor.matmul(
                pos[j], lhsT=vnat[:, kb, 0:DIMS + 1], rhs=at[:, j, :],
                start=(kb == 0), stop=(kb == KBLKS - 1),
            )
    # --- output: transpose oT blocks, normalize, store (per q tile, so the
    # first half of the output drains while the second is still accumulating)
    for j in range(2):
        nc.vector.tensor_copy(oT[:, j * 512:(j + 1) * 512], pos[j])
        for ob in range(j * OBLKS // 2, (j + 1) * OBLKS // 2):
            pt = pt_pool.tile([128, DIMS + 1], F32, tag="pt")
            nc.tensor.transpose(
                pt, oT[:, ob * 128:(ob + 1) * 128], ident[0:DIMS + 1, 0:DIMS + 1]
            )
            rcp = outs.tile([128, 1], F32, tag="rcp")
            nc.vector.reciprocal(rcp, pt[:, DIMS:DIMS + 1])
            ob_sb = outs.tile([128, DIMS], F32, tag="ob")
            nc.vector.tensor_scalar_mul(ob_sb, pt[:, 0:DIMS], rcp)
            nc.sync.dma_start(out=out[ob * 128:(ob + 1) * 128, :], in_=ob_sb)


_NC_CACHE = None


def _build():
    global _NC_CACHE
    if _NC_CACHE is not None:
        return _NC_CACHE
    nc = bacc.Bacc(
        "TRN2", target_bir_lowering=False, debug=False,
        enable_asserts=True, num_devices=N_CORES,
    )
    encT = nc.dram_tensor("encT", [D, SK], BF16, kind="ExternalInput").ap()
    decT = nc.dram_tensor("decT", [D, SQ], BF16, kind="ExternalInput").ap()
    wkv = nc.dram_tensor("wkv", [D, 128], BF16, kind="ExternalInput").ap()
    wq = nc.dram_tensor("wq", [D, DIMS], BF16, kind="ExternalInput").ap()
    bv = nc.dram_tensor("bv", [DIMS, 1], F32, kind="ExternalInput").ap()
    bk = nc.dram_tensor("bk", [DIMS, 1], F32, kind="ExternalInput").ap()
    bq = nc.dram_tensor("bq", [DIMS, 1], F32, kind="ExternalInput").ap()
    out = nc.dram_tensor("out", [SQ, DIMS], F32, kind="ExternalOutput").ap()
    with tile.TileContext(nc) as tc:
        _body(tc, encT, decT, wkv, wq, bv, bk, bq, out)
    nc.compile()
    _NC_CACHE = nc
    return nc


def make_in_maps(**inputs):
    bf16 = ml_dtypes.bfloat16
    enc = np.asarray(inputs["encoder_output"])
    dec = np.asarray(inputs["decoder"])
    scale = DIMS ** -0.5
    wq_s = (np.asarray(inputs["Wq"]) * scale).astype(bf16)
    bq_s = (np.asarray(inputs["bq"]) * scale).astype(np.float32).reshape(DIMS, 1)
    wkv = np.concatenate(
        [np.asarray(inputs["Wv"]), np.asarray(inputs["Wk"])], axis=1
    ).astype(bf16)
    bv = np.asarray(inputs["bv"]).astype(np.float32).reshape(DIMS, 1)
    bk = np.asarray(inputs["bk"]).astype(np.float32).reshape(DIMS, 1)
    in_maps = []
    for c in range(N_CORES):
        b, h = divmod(c, 2)
        in_maps.append({
            "encT": enc[b].T.astype(bf16),
            "decT": dec[b, h * SQ:(h + 1) * SQ, :].T.astype(bf16),
            "wkv": wkv, "wq": wq_s, "bv": bv, "bk": bk, "bq": bq_s,
        })
    return in_maps


def assemble(results):
    out = np.zeros((B, SQ_FULL, DIMS), np.float32)
    for c in range(N_CORES):
        b, h = divmod(c, 2)
        out[b, h * SQ:(h + 1) * SQ] = results[c]["out"]
    return out


def kernel(**inputs) -> np.ndarray:
    nc = _build()
    in_maps = make_in_maps(**inputs)
    res = run_bass_kernel_spmd(nc, in_maps, core_ids=list(range(N_CORES)))
    return assemble(res.results)


# revision 14
# speedup vs baseline: 1.2365x; 1.2365x over previous
"""Cross-attention kernel for Trainium2, distributed over 8 NeuronCores.

Problem: B=4, Sk=4096, Sq=2048, d_model=1024, dims=64 (fp32 reference).

Sharding (hardcoded): core c -> (batch b = c//2, decoder half h = c%2).
Each core computes out[b, h*1024:(h+1)*1024, :] from enc[b] and its decoder
slice. No collectives.

Per-core dataflow (all layouts chosen so no large on-chip transposes are
needed):
  - Host pre-transposes/casts activations to bf16: encT [1024,4096],
    decT [1024,1024] (d_model on partitions).
  - KV^T projection: lhsT = [Wv | Wk] [128d, 128], rhs = encT chunks
    -> psum [128, 512] where rows 0:64 = V^T, 64:128 = K^T. Full PE array.
  - V^T is evacuated into an 80-partition tile whose row 64 is constant 1.0;
    a DMA x-bar transpose then yields V-natural blocks [128k, 80] whose
    col 64 is the ones column -> AV lhsT [128, 65] directly, and the ones
    column accumulates the softmax denominator during the AV matmul.
  - Scores computed transposed: S^T[k,q] = (K Q^T); exp(S^T) on ACT
    (PSUM->SBUF bf16) feeds the AV matmul as the moving operand. No
    max-subtraction (|scores| ~ N(0,1), exp is safe in fp32).
  - out^T [65, q] accumulated in PSUM over k blocks; final transpose of the
    small output via PE + per-partition reciprocal-scale on DVE.
"""

import numpy as np
import ml_dtypes

import concourse.bass as bass
import concourse.bacc as bacc
import concourse.tile as tile
from concourse import mybir
from concourse._compat import with_exitstack
from concourse.bass_utils import run_bass_kernel_spmd
from concourse.masks import make_identity

BF16 = mybir.dt.bfloat16
F32 = mybir.dt.float32

B, SK, SQ_FULL, D, DIMS = 4, 4096, 2048, 1024, 64
N_CORES = 8
SQ = SQ_FULL * B // N_CORES  # 1024 decoder rows per core
DC = D // 128  # d_model chunks of 128
KPAIRS = SK // 1024  # 4 enc column pair-tiles
KBLKS = SK // 128  # 32 k blocks for attention
OBLKS = SQ // 128  # 8 output row blocks


@with_exitstack
def _body(ctx, tc, encT, decT, wkv, wq, bv, bk, bq, out):
    nc = tc.nc

    singles = ctx.enter_context(tc.tile_pool(name="singles", bufs=1))
    loads = ctx.enter_context(tc.tile_pool(name="loads", bufs=3))
    ps_pool = ctx.enter_context(tc.tile_pool(name="ps", bufs=2, space="PSUM"))
    po_pool = ctx.enter_context(tc.tile_pool(name="po", bufs=2, space="PSUM"))
    pt_pool = ctx.enter_context(tc.tile_pool(name="pt", bufs=2, space="PSUM"))
    at_pool = ctx.enter_context(tc.tile_pool(name="at", bufs=4))
    outs = ctx.enter_context(tc.tile_pool(name="outs", bufs=3))

    # --- constants (small loads on the SWDGE/gpsimd queue, so the big
    # activation streams own the HWDGE path) ---
    wkv_sb = singles.tile([128, DC, 128], BF16)
    nc.gpsimd.dma_start(out=wkv_sb, in_=wkv.rearrange("(c p) m -> p c m", p=128))
    wq_sb = singles.tile([128, DC, DIMS], BF16)
    nc.gpsimd.dma_start(out=wq_sb, in_=wq.rearrange("(c p) m -> p c m", p=128))
    bv_sb = singles.tile([DIMS, 1], F32)
    nc.gpsimd.dma_start(out=bv_sb, in_=bv)
    bk_sb = singles.tile([DIMS, 1], F32)
    nc.gpsimd.dma_start(out=bk_sb, in_=bk)
    bq_sb = singles.tile([DIMS, 1], F32)
    nc.gpsimd.dma_start(out=bq_sb, in_=bq)
    ident = singles.tile([128, 128], F32)
    make_identity(nc, ident)

    # --- persistent activations ---
    # K=64 weight loads cannot overlap an in-flight matmul streaming the same
    # PE-array rows (no FWL shadow for 64-row stationaries), so consecutive
    # S matmuls must alternate array quadrants. K^T and the q operands are
    # duplicated on both partition halves; each S matmul picks the half its
    # predecessor is not using.
    kTd = singles.tile([128, SK], BF16)
    # V^T with a ones row baked in at row 64 (rows 65:80 are never consumed;
    # partition count must be 16-aligned for the x-bar transpose).
    vTx = singles.tile([80, SK], BF16)
    nc.gpsimd.memset(vTx[64:65, :], 1.0)
    # V natural blocks: vnat[p, c, 0:64] = V[c*128+p, :], col 64 = 1.0
    vnat = singles.tile([128, KBLKS, 80], BF16)
    qTd = singles.tile([128, SQ], BF16)
    oT = singles.tile([DIMS + 1, SQ], F32)

    # --- activation loads: 1 MB chunks on HWDGE, ordered so the first
    # KV pair and the decoder land first ---
    esbs = []
    for kp in range(KPAIRS):
        e0 = loads.tile([128, 4, 1024], BF16, tag="eload", name=f"esb{kp}a")
        e1 = loads.tile([128, 4, 1024], BF16, tag="eload", name=f"esb{kp}b")
        esbs.append((e0, e1))
    dsb = loads.tile([128, DC, SQ], BF16, tag="dload")

    def load_pair(kp):
        enc_r = encT.rearrange("(c p) n -> p c n", p=128)
        sl = slice(kp * 1024, (kp + 1) * 1024)
        nc.sync.dma_start(out=esbs[kp][0], in_=enc_r[:, 0:4, sl])
        nc.sync.dma_start(out=esbs[kp][1], in_=enc_r[:, 4:8, sl])

    load_pair(0)
    dec_r = decT.rearrange("(c p) n -> p c n", p=128)
    nc.sync.dma_start(out=dsb[:, 0:4, :], in_=dec_r[:, 0:4, :])
    nc.sync.dma_start(out=dsb[:, 4:8, :], in_=dec_r[:, 4:8, :])
    for kp in range(1, KPAIRS):
        load_pair(kp)

    def kv_pair(kp):
        pskv = ps_pool.tile([128, 2, 512], F32, tag="ps", name=f"pskv{kp}")
        for d in range(DC):
            esb = esbs[kp][d // 4]
            for j in range(2):
                nc.tensor.matmul(
                    pskv[:, j, :], lhsT=wkv_sb[:, d, :],
                    rhs=esb[:, d % 4, j * 512:(j + 1) * 512],
                    start=(d == 0), stop=(d == DC - 1),
                )
        for j in range(2):
            sl = slice(kp * 1024 + j * 512, kp * 1024 + (j + 1) * 512)
            nc.vector.tensor_scalar_add(vTx[0:DIMS, sl], pskv[0:DIMS, j, :], bv_sb)
            nc.vector.tensor_scalar_add(kTd[0:DIMS, sl], pskv[DIMS:128, j, :], bk_sb)
            nc.vector.tensor_scalar_add(kTd[DIMS:128, sl], pskv[DIMS:128, j, :], bk_sb)
        # V natural layout via one x-bar transpose per 1024 columns
        nc.sync.dma_start_transpose(
            out=vnat[:, kp * 8:(kp + 1) * 8, :],
            in_=vTx[:, kp * 1024:(kp + 1) * 1024],
        )

    # --- K^T / V^T projection (packed): rows 0:64 = V^T, 64:128 = K^T ---
    kv_pair(0)

    # --- Q^T projection: qT[64, SQ] = Wq^T @ decT ---
    psq = ps_pool.tile([DIMS, 2, 512], F32, tag="ps")
    for d in range(DC):
        for j in range(2):
            nc.tensor.matmul(
                psq[:, j, :], lhsT=wq_sb[:, d, :], rhs=dsb[:, d, j * 512:(j + 1) * 512],
                start=(d == 0), stop=(d == DC - 1),
            )
    for j in range(2):
        nc.vector.tensor_scalar_add(
            qTd[0:DIMS, j * 512:(j + 1) * 512], psq[:, j, :], bq_sb
        )
        nc.vector.tensor_scalar_add(
            qTd[DIMS:128, j * 512:(j + 1) * 512], psq[:, j, :], bq_sb
        )

    for kp in range(1, KPAIRS):
        kv_pair(kp)

    # --- attention: S^T = K Q^T per k block (both q tiles share the
    #     stationary operand); exp; accumulate [V|1]^T @ exp(S^T) ---
    po0 = po_pool.tile([DIMS + 1, 512], F32, tag="po")
    po1 = po_pool.tile([DIMS + 1, 512], F32, tag="po")
    pos = [po0, po1]
    # process k blocks in pairs: a run of 4 S matmuls (quadrant-alternating),
    # the two exps, then a run of 4 AV matmuls — long uninterrupted PE streams
    for kg in range(KBLKS // 2):
        psses = []
        ats = []
        for kb in (2 * kg, 2 * kg + 1):
            pss = ps_pool.tile([128, 2, 512], F32, tag="ps", name=f"pss{kb % 2}")
            psses.append(pss)
            for j in range(2):
                hp = DIMS * j
                nc.tensor.matmul(
                    pss[:, j, :], lhsT=kTd[hp:hp + DIMS, kb * 128:(kb + 1) * 128],
                    rhs=qTd[hp:hp + DIMS, j * 512:(j + 1) * 512],
                    start=True, stop=True,
                )
        for i, kb in enumerate((2 * kg, 2 * kg + 1)):
            at = at_pool.tile([128, 2, 512], BF16, tag="at", name=f"at{kb % 2}")
            ats.append(at)
            nc.scalar.activation(at, psses[i], mybir.ActivationFunctionType.Exp)
        for i, kb in enumerate((2 * kg, 2 * kg + 1)):
            for j in range(2):
                nc.tensor.matmul(
                    pos[j], lhsT=vnat[:, kb, 0:DIMS + 1], rhs=ats[i][:, j, :],
                    start=(kb == 0), stop=(kb == KBLKS - 1),
                )
    # --- output: transpose oT blocks, normalize, store (per q tile, so the
    # first half of the output drains while the second is still accumulating)
    for j in range(2):
        nc.vector.tensor_copy(oT[:, j * 512:(j + 1) * 512], pos[j])
        for ob in range(j * OBLKS // 2, (j + 1) * OBLKS // 2):
            pt = pt_pool.tile([128, DIMS + 1], F32, tag="pt")
            nc.tensor.transpose(
                pt, oT[:, ob * 128:(ob + 1) * 128], ident[0:DIMS + 1, 0:DIMS + 1]
            )
            rcp = outs.tile([128, 1], F32, tag="rcp")
            nc.vector.reciprocal(rcp, pt[:, DIMS:DIMS + 1])
            ob_sb = outs.tile([128, DIMS], F32, tag="ob")
            nc.vector.tensor_scalar_mul(ob_sb, pt[:, 0:DIMS], rcp)
            nc.sync.dma_start(out=out[ob * 128:(ob + 1) * 128, :], in_=ob_sb)


_NC_CACHE = None


def _build():
    global _NC_CACHE
    if _NC_CACHE is not None:
        return _NC_CACHE
    nc = bacc.Bacc(
        "TRN2", target_bir_lowering=False, debug=False,
        enable_asserts=True, num_devices=N_CORES,
    )
    encT = nc.dram_tensor("encT", [D, SK], BF16, kind="ExternalInput").ap()
    decT = nc.dram_tensor("decT", [D, SQ], BF16, kind="ExternalInput").ap()
    wkv = nc.dram_tensor("wkv", [D, 128], BF16, kind="ExternalInput").ap()
    wq = nc.dram_tensor("wq", [D, DIMS], BF16, kind="ExternalInput").ap()
    bv = nc.dram_tensor("bv", [DIMS, 1], F32, kind="ExternalInput").ap()
    bk = nc.dram_tensor("bk", [DIMS, 1], F32, kind="ExternalInput").ap()
    bq = nc.dram_tensor("bq", [DIMS, 1], F32, kind="ExternalInput").ap()
    out = nc.dram_tensor("out", [SQ, DIMS], F32, kind="ExternalOutput").ap()
    with tile.TileContext(nc) as tc:
        _body(tc, encT, decT, wkv, wq, bv, bk, bq, out)
    nc.compile()
    _NC_CACHE = nc
    return nc


def make_in_maps(**inputs):
    bf16 = ml_dtypes.bfloat16
    enc = np.asarray(inputs["encoder_output"])
    dec = np.asarray(inputs["decoder"])
    scale = DIMS ** -0.5
    wq_s = (np.asarray(inputs["Wq"]) * scale).astype(bf16)
    bq_s = (np.asarray(inputs["bq"]) * scale).astype(np.float32).reshape(DIMS, 1)
    wkv = np.concatenate(
        [np.asarray(inputs["Wv"]), np.asarray(inputs["Wk"])], axis=1
    ).astype(bf16)
    bv = np.asarray(inputs["bv"]).astype(np.float32).reshape(DIMS, 1)
    bk = np.asarray(inputs["bk"]).astype(np.float32).reshape(DIMS, 1)
    in_maps = []
    for c in range(N_CORES):
        b, h = divmod(c, 2)
        in_maps.append({
            "encT": enc[b].T.astype(bf16),
            "decT": dec[b, h * SQ:(h + 1) * SQ, :].T.astype(bf16),
            "wkv": wkv, "wq": wq_s, "bv": bv, "bk": bk, "bq": bq_s,
        })
    return in_maps


def assemble(results):
    out = np.zeros((B, SQ_FULL, DIMS), np.float32)
    for c in range(N_CORES):
        b, h = divmod(c, 2)
        out[b, h * SQ:(h + 1) * SQ] = results[c]["out"]
    return out


def kernel(**inputs) -> np.ndarray:
    nc = _build()
    in_maps = make_in_maps(**inputs)
    res = run_bass_kernel_spmd(nc, in_maps, core_ids=list(range(N_CORES)))
    return assemble(res.results)
